# revision 32
# baseline (speedup 1.0000x reference)
"""Multi-head attention (B=8, N=1024, C=768, H=12) on 8 TRN2 NeuronCores.

Sharding: data-parallel - one batch element per core, weights replicated.
No collectives.

Design (flat-pipeline rev, from ~180us baseline): bf16 matmul operands
(f32 PSUM), the two K=64 S^T matmuls row-tile into PE halves and run
concurrently (auto tile_position from base_partition 0/64). The whole
attention is ONE software-pipelined stream over 96 (pair, half, jtile)
steps: S/exp run one step ahead of PV, flowing across block boundaries,
so the ACT engine (exp, (1024+352)/1.2 = 1.15us per tile, 110us total -
the real floor) never waits at a boundary. V generation runs up front
(DMA order xt, wv, wq, wk, wp) replacing most of the dummy-warmup
window, so pair 0 needs no deferred-PV special case. QK chains for pair
t+1 drain as slack-sized fillers inside pair t. Reciprocal via the fast
custom-DVE approx, partition broadcast on GPSIMD, input DMAs split
two-ways per tensor on the Sync queue.

Per-core dataflow:
  qt/kt [128, pair, 1024]: rows = head-pair dims (A at 0:64, B at 64:128).
  v_sb [128 j, jt, head, 65]: col 64 is ones -> PV row 64 = softmax denom.
  Per step: S^T halves via two K=64 row-tiled matmuls (concurrent), one
  exp ACTIVATE (FD=1024, scale fused), PV accumulates O^T[65, 512].
  Normalize: denom row -> reciprocal_approx_fast -> partition_broadcast
  -> multiply into ot_sb (bf16).
  Proj: y = OT.T @ wp + bias per 128-row tile, straight to DRAM.
"""

from contextlib import ExitStack

import numpy as np

import concourse.bacc as bacc
import concourse.mybir as mybir
import concourse.tile as tile
from concourse.bass_utils import run_bass_kernel_spmd

F32 = mybir.dt.float32
BF16 = mybir.dt.bfloat16

B, N, C = 8, 1024, 768
H, HD = 12, 64
SCALE = HD ** -0.5
NT_I = N // 128   # 8 i/j tiles
NT_C = C // 128   # 6 c tiles (== head pairs)
NPAIR = H // 2    # 6
N_WARM = 18


def build():
    nc = bacc.Bacc(None, target_bir_lowering=False)

    # inputs are host-pre-arranged to the exact SBUF layout
    # [128 partitions, k-tile, free] AND pre-split into k-halves as
    # separate dram tensors, so every DMA descriptor is a fully
    # contiguous 4.6-9KB run per partition line. (A dram-side strided
    # slice of one 3D tensor corrupts on HW - multi-queue transfer vs
    # completion-sem race.)
    h = NT_C // 2
    ins = {}
    for nm, fd in (("xt", N), ("wq", C), ("wk", C), ("wv", C), ("wp", C)):
        ins[nm] = [nc.dram_tensor(f"{nm}{i}", [128, h, fd], BF16,
                                  kind="ExternalInput") for i in range(2)]
    bias = nc.dram_tensor("bias", [128, C], F32, kind="ExternalInput")
    # y in bf16: halves the output-DMA bytes on the kernel tail; the
    # host casts back to fp32 (quantization ~1.4e-3 abs vs 2e-2 budget)
    y = nc.dram_tensor("y", [N, C], BF16, kind="ExternalOutput")

    with tile.TileContext(nc) as tc, ExitStack() as stack:
        pp = stack.enter_context(tc.tile_pool(name="persist", bufs=1))
        p_pt = stack.enter_context(tc.tile_pool(name="pt", bufs=16))
        p_nrm = stack.enter_context(tc.tile_pool(name="nrm", bufs=4))
        p_y = stack.enter_context(tc.tile_pool(name="yout", bufs=8))
        ps_qkv = stack.enter_context(
            tc.tile_pool(name="psq", bufs=2, space="PSUM"))
        ps_st = stack.enter_context(
            tc.tile_pool(name="psst", bufs=1, space="PSUM"))
        ps_ov = stack.enter_context(
            tc.tile_pool(name="psov", bufs=1, space="PSUM"))

        xt_sb = pp.tile([128, NT_C, N], BF16)
        wq_sb = pp.tile([128, NT_C, C], BF16)
        wk_sb = pp.tile([128, NT_C, C], BF16)
        wv_sb = pp.tile([128, NT_C, C], BF16)
        wp_sb = pp.tile([128, NT_C, C], BF16)
        bias_sb = pp.tile([128, C], F32)
        qt_sb = pp.tile([128, NPAIR, N], BF16)
        kt_sb = pp.tile([128, NPAIR, N], BF16)
        v_sb = pp.tile([128, NT_I, H, HD + 1], BF16)
        ot_sb = pp.tile([128, NPAIR, N], BF16)

        # Input DMAs: descriptor issue on the Sync queue serializes the
        # transfer STARTS (~1us per descriptor, recycled-sem chaining),
        # capping effective input bandwidth ~210GB/s. Fan the four
        # critical tensors across four engine queues so their transfers
        # genuinely overlap; wp/bias trail on sync (not needed until the
        # projection tail).
        def dma_k2(eng, dst, srcs):
            eng.dma_start(dst[:, 0:h], srcs[0][:])
            eng.dma_start(dst[:, h:NT_C], srcs[1][:])

        # xt/wv halves interleaved: V's first k-steps need only
        # (xt-h1, wv-h1), so V generation starts at ~13.5us instead of
        # ~17.5 and the whole V phase finishes while wq/wk stream in.
        nc.sync.dma_start(xt_sb[:, 0:h], ins["xt"][0][:])
        nc.sync.dma_start(wv_sb[:, 0:h], ins["wv"][0][:])
        nc.sync.dma_start(xt_sb[:, h:NT_C], ins["xt"][1][:])
        nc.sync.dma_start(wv_sb[:, h:NT_C], ins["wv"][1][:])
        dma_k2(nc.sync, wq_sb, ins["wq"])
        dma_k2(nc.sync, wk_sb, ins["wk"])
        dma_k2(nc.sync, wp_sb, ins["wp"])
        nc.sync.dma_start(bias_sb[:], bias[:])

        # HAM warmup: keep the PE busy from the end of the engine preamble
        # until xt+wv have streamed in and real V chains take over.
        warm_a = pp.tile([128, 128], BF16)
        warm_b = pp.tile([128, 512], BF16)
        nc.vector.memset(warm_a[:], 0.0)
        nc.vector.memset(warm_b[:], 0.0)
        for w in range(N_WARM):
            warm_ps = ps_st.tile([128, 512], F32, tag=f"st{w % 2}",
                                 name=f"warm{w}")
            nc.tensor.matmul(warm_ps[:], warm_a[:], warm_b[:])
        nc.vector.memset(v_sb[:, :, :, HD:HD + 1], 1.0)

        def gen_qk_chunk(t, which, ch, ks=None, acc=None):
            """One accumulation chain of Q.T (which=0) or K.T (which=1).
            With ks given, emits only those k-steps of the chain (the
            caller threads the acc tile through) - used to granularize
            filler work to the per-step PE slack."""
            w_sb, out_sb = ((wq_sb, qt_sb), (wk_sb, kt_sb))[which]
            if acc is None:
                acc = ps_qkv.tile([128, 512], F32, tag="acc",
                                  name=f"qk{t}_{which}_{ch}")
            for k in (ks if ks is not None else range(NT_C)):
                nc.tensor.matmul(
                    acc[:],
                    w_sb[:, k, t * 128:(t + 1) * 128],
                    xt_sb[:, k, ch * 512:(ch + 1) * 512],
                    start=(k == 0), stop=(k == NT_C - 1),
                )
            if ks is None or ks[-1] == NT_C - 1:
                nc.vector.tensor_copy(out_sb[:, t, ch * 512:(ch + 1) * 512],
                                      acc[:])
            return acc

        def qk_halves(t, which, ch):
            """Two filler pieces continuing one accumulation chain."""
            state = {}

            def first():
                state["acc"] = gen_qk_chunk(t, which, ch, ks=[0, 1, 2])

            def second():
                gen_qk_chunk(t, which, ch, ks=[3, 4, 5], acc=state["acc"])

            return [first, second]

        def gen_v_chunk(jt, ch):
            acc = ps_qkv.tile([128, 384], F32, tag="acc",
                              name=f"v{jt}_{ch}")
            for k in range(NT_C):
                nc.tensor.matmul(
                    acc[:],
                    xt_sb[:, k, jt * 128:(jt + 1) * 128],
                    wv_sb[:, k, ch * 384:(ch + 1) * 384],
                    start=(k == 0), stop=(k == NT_C - 1),
                )
            nc.vector.tensor_copy(
                v_sb[:, jt, 6 * ch:6 * ch + 6, 0:HD],
                acc[:].rearrange("p (h e) -> p h e", e=HD),
            )

        # ---- flat software-pipelined attention ------------------------
        # blocks: (pair t, query half ib), 8 j-steps each. Global step g:
        # S+exp for step g+1 are emitted before PV of step g, across
        # block boundaries, so ACT never drains at a boundary. ov PSUM
        # tags A/B pin per-block slot reuse; st tags alternate by global
        # step parity so S(g) waits only exp(g-2).
        blocks = [(t, ib) for ib in range(2) for t in range(NPAIR)]
        NB = len(blocks)

        def s_exp(bi, n, g):
            t, ib = blocks[bi]
            i0 = ib * 512
            st = ps_st.tile([128, 1024], F32, tag=f"st{g % 2}",
                            name=f"st{t}_{ib}_{n}")
            nc.tensor.matmul(
                st[:, 0:512],
                kt_sb[0:64, t, n * 128:(n + 1) * 128],
                qt_sb[0:64, t, i0:i0 + 512],
            )
            nc.tensor.matmul(
                st[:, 512:1024],
                kt_sb[64:128, t, n * 128:(n + 1) * 128],
                qt_sb[64:128, t, i0:i0 + 512],
            )
            pt = p_pt.tile([128, 1024], BF16, tag="pt")
            nc.scalar.activation(
                pt[:], st[:],
                mybir.ActivationFunctionType.Exp, scale=SCALE,
            )
            return pt

        ovs = {}

        def pv(bi, n, pt):
            t, ib = blocks[bi]
            hA, hB = 2 * t, 2 * t + 1
            if n == 0:
                ovs[bi] = (
                    ps_ov.tile([HD + 1, 512], F32, tag="ovA",
                               name=f"ovA{t}_{ib}"),
                    ps_ov.tile([HD + 1, 512], F32, tag="ovB",
                               name=f"ovB{t}_{ib}"),
                )
            ovA, ovB = ovs[bi]
            nc.tensor.matmul(
                ovA[:], v_sb[:, n, hA, :], pt[:, 0:512],
                start=(n == 0), stop=(n == NT_I - 1),
            )
            nc.tensor.matmul(
                ovB[:], v_sb[:, n, hB, :], pt[:, 512:1024],
                start=(n == 0), stop=(n == NT_I - 1),
            )

        def normalize(bi):
            t, ib = blocks[bi]
            i0 = ib * 512
            ovA, ovB = ovs.pop(bi)
            # copy the unnormalized O^T + denom row out first so the ov
            # banks free right after the last PV; the multiply runs
            # in-place in SBUF whenever DVE has slack. (Keep every op
            # here off GPSIMD except the broadcast: any gpsimd tensor op
            # forces an UNLOAD_LIB/LOAD_LIB swap around the custom
            # partition_broadcast library - measured ~8us of serialized
            # lib churn in the tail.)
            for base, ov in ((0, ovA), (64, ovB)):
                osl = ot_sb[base:base + 64, t, i0:i0 + 512]
                rl = p_nrm.tile([1, 512], F32, tag="rl")
                rc = p_nrm.tile([1, 512], F32, tag="rc")
                bc = p_nrm.tile([128, 512], F32, tag="bc")
                nc.vector.tensor_copy(rl[0:1, :], ov[64:65, :])
                nc.vector.tensor_copy(osl, ov[0:64, :])
                nc.vector.reciprocal_approx_fast(rc[0:1, :], rl[0:1, :])
                nc.gpsimd.partition_broadcast(bc[:], rc[0:1, :])
                nc.vector.tensor_mul(osl, osl, bc[base:base + 64, :])

        y_part = pp.tile([128, 4, C], F32)

        def proj(it, ks, first=True, last=True):
            """Projection of row-tile it over c_in chunks ks. Row-tiles
            4-7 run after the last exp, so their accumulators use the
            then-free st banks - 4 chains in flight instead of 2."""
            y_sb = (p_y.tile([128, C], BF16, tag="y", name=f"y{it}")
                    if last else None)
            for ch in range(2):
                if it >= 4:
                    pool, tag = ((ps_st, "st0"), (ps_st, "st1"),
                                 (ps_ov, "ovA"), (ps_ov, "ovB"))[
                                     (it % 2) * 2 + ch]
                    acc = pool.tile([128, 384], F32, tag=tag,
                                    name=f"p{it}_{ks[0]}_{ch}")
                else:
                    acc = ps_qkv.tile([128, 384], F32, tag="acc",
                                      name=f"p{it}_{ks[0]}_{ch}")
                for k in ks:
                    nc.tensor.matmul(
                        acc[:],
                        ot_sb[:, k, it * 128:(it + 1) * 128],
                        wp_sb[:, k, ch * 384:(ch + 1) * 384],
                        start=(k == ks[0]), stop=(k == ks[-1]),
                    )
                sl = slice(ch * 384, (ch + 1) * 384)
                prev = bias_sb if first else y_part[:, it - 4, :]
                dst = y_sb if last else y_part[:, it - 4, :]
                nc.vector.tensor_add(dst[:, sl], acc[:], prev[:, sl])
            if last:
                eng = (nc.sync, nc.gpsimd)[it % 2]
                eng.dma_start(y[it * 128:(it + 1) * 128, :], y_sb[:])

        # Prologue: the first exp is gated on Q-ch0 + K-ch0, which are
        # gated on wq/wk (DMA order: xt, wv, wq, wk) - emit them right
        # after the V chains that cover the wv->wq DMA window, and push
        # the rest of V behind them (V[jt] is only read by PV step jt,
        # 1+jt exp-steps after the first exp). V jt 4-7 ride as pair-0
        # fillers.
        for jt in range(6):
            for ch in range(2):
                gen_v_chunk(jt, ch)
        gen_qk_chunk(0, 0, 0)
        gen_qk_chunk(0, 1, 0)

        # fillers (ib-major): K ch0+ch1 and Q ch0 for pair t+1 are due
        # at block (t+1,0) - three chains spread inside block (t,0). Q
        # ch1 for pair t rides ~6 blocks ahead of its (t,1) block. proj
        # row-tiles 0-3 (ready once (5,0) normalizes) prefill the ib=1
        # phase; the tail is just row-tiles 4-7.
        fillers = {}
        for bi, (t, ib) in enumerate(blocks):
            f = []
            lo = 1
            if ib == 0 and t == 0:
                f.append(lambda: gen_qk_chunk(0, 1, 1))
                f += [lambda jt=jt, ch=ch: gen_v_chunk(jt, ch)
                      for jt in (6, 7) for ch in range(2)]
                f.append(lambda: gen_qk_chunk(1, 1, 0))
                f.append(lambda: gen_qk_chunk(1, 1, 1))
                f.append(lambda: gen_qk_chunk(1, 0, 0))
                lo = 0
            elif ib == 0 and t + 1 < NPAIR:
                f.append(lambda t=t: gen_qk_chunk(t + 1, 1, 0))
                f.append(lambda t=t: gen_qk_chunk(t + 1, 1, 1))
                f.append(lambda t=t: gen_qk_chunk(t + 1, 0, 0))
            elif ib == 0:  # (5,0)
                f.append(lambda: gen_qk_chunk(0, 0, 1))
                lo = 2
            elif t + 1 < NPAIR:  # (t,1), t<5
                f.append(lambda t=t: gen_qk_chunk(t + 1, 0, 1))
                if t < 4:
                    f.append(lambda t=t: proj_ch(t, 0))
                    f.append(lambda t=t: proj_ch(t, 1))
                lo = 2
            fillers[bi] = (lo, f)

        proj_accs = {}

        def proj_ch(it, ch):
            """One channel of proj row-tile it (prefill, full k chain)."""
            acc = ps_qkv.tile([128, 384], F32, tag="acc",
                              name=f"p{it}_{ch}")
            for k in range(NT_C):
                nc.tensor.matmul(
                    acc[:],
                    ot_sb[:, k, it * 128:(it + 1) * 128],
                    wp_sb[:, k, ch * 384:(ch + 1) * 384],
                    start=(k == 0), stop=(k == NT_C - 1),
                )
            if ch == 0:
                proj_accs[it] = p_y.tile([128, C], BF16, tag="y",
                                         name=f"y{it}")
            y_sb = proj_accs[it]
            sl = slice(ch * 384, (ch + 1) * 384)
            nc.vector.tensor_add(y_sb[:, sl], acc[:], bias_sb[:, sl])
            if ch == 1:
                eng = (nc.sync, nc.gpsimd)[it % 2]
                eng.dma_start(y[it * 128:(it + 1) * 128, :], y_sb[:])

        pts = {}
        for g in range(NB * NT_I + 1):
            if g < NB * NT_I:
                bi, n = divmod(g, NT_I)
                pts[g] = s_exp(bi, n, g)
            if g >= 1:
                bi_p, n_p = divmod(g - 1, NT_I)
                pv(bi_p, n_p, pts.pop(g - 1))
                if n_p == NT_I - 1:
                    normalize(bi_p)
            if g < NB * NT_I:
                # Drain cadence: 4-piece blocks drain at n=2..5, keeping
                # the fillers' PSUM-acc CASTs off the DVE while it runs
                # the previous block's normalize (n=0..1) - otherwise the
                # chain-start matmuls stall on acc slots (~1us per pair
                # boundary). 8-piece blocks need every slot. Last block:
                # n>=4 so pair-5-ib0's normalize mul (read by the proj
                # prefill's k=5) lands first.
                lo, f = fillers[bi]
                if f and n >= lo:
                    f.pop(0)()
        for bi in range(NB):
            while fillers[bi][1]:
                fillers[bi][1].pop(0)()

        # projection tail: only row-tiles 4-7 remain (0-3 prefilled in
        # the ib=1 phase). k=0..4 chains first - they don't depend on
        # the last normalize, whose recip/broadcast/mul only k=5 reads.
        for it in (4, 5, 6, 7):
            proj(it, [0, 1, 2, 3, 4], first=True, last=False)
        for it in (4, 5, 6, 7):
            proj(it, [5], first=False, last=True)

    nc.compile()
    nc.finalize()
    return nc


_NC_CACHE = {}


def _get_nc(mode=None):
    if "nc" not in _NC_CACHE:
        _NC_CACHE["nc"] = build()
    return _NC_CACHE["nc"]


def _prep_host(x, w_qkv, w_proj, b_proj, mode=None):
    import ml_dtypes
    bf16 = ml_dtypes.bfloat16

    x = np.asarray(x)
    w_qkv = np.asarray(w_qkv)
    w_proj = np.asarray(w_proj)
    b_proj = np.asarray(b_proj)
    h = NT_C // 2

    def arr(a):  # [C, F] -> SBUF-layout k-halves 2x[128, h, F]
        b = np.ascontiguousarray(
            a.reshape(NT_C, 128, a.shape[1]).transpose(1, 0, 2)).astype(bf16)
        return (np.ascontiguousarray(b[:, 0:h]),
                np.ascontiguousarray(b[:, h:NT_C]))

    xt = [arr(x[b].T) for b in range(B)]  # per-core ([128,h,N], [128,h,N])
    wq_t = arr(w_qkv[0:C].T)
    wk_t = arr(w_qkv[C:2 * C].T)
    wv_t = arr(w_qkv[2 * C:3 * C].T)
    wp_t = arr(w_proj.T)
    bias_rep = np.ascontiguousarray(
        np.broadcast_to(np.asarray(b_proj, dtype=np.float32), (128, C)))
    return xt, wq_t, wk_t, wv_t, wp_t, bias_rep


def _make_in_maps(x, w_qkv, w_proj, b_proj, mode=None):
    xt, wq_t, wk_t, wv_t, wp_t, bias_rep = _prep_host(x, w_qkv, w_proj, b_proj)
    in_maps = []
    for b in range(B):
        m = {"bias": bias_rep}
        for nm, val in (("xt", xt[b]), ("wq", wq_t), ("wk", wk_t),
                        ("wv", wv_t), ("wp", wp_t)):
            m[f"{nm}0"], m[f"{nm}1"] = val
        in_maps.append(m)
    return in_maps


def run(x, w_qkv, w_proj, b_proj, mode=None, trace=False):
    nc = _get_nc()
    in_maps = _make_in_maps(x, w_qkv, w_proj, b_proj)
    res = run_bass_kernel_spmd(
        nc, in_maps, core_ids=list(range(B)), trace=trace
    )
    out = np.stack([res.results[b]["y"] for b in range(B)]).astype(np.float32)
    return out, res


def kernel(x, w_qkv, w_proj, b_proj):
    out, _ = run(x, w_qkv, w_proj, b_proj)
    return out
